# revision 20
# baseline (speedup 1.0000x reference)
"""DiscretizedMixLogisticLoss Bass kernel for TRN2, 8-core data-parallel. v7.

Full inputs: x [8,3,256,256] f32, l [8,120,256,256] f32 -> nll [8,3,256,256] f32.
Sharding: batch dim N=8 across 8 cores (1 example/core).

Math (per pixel, channel c, mixture k), with l viewed as [4,3,10,H*W]:
  s = l[0], mu = l[1], sc = l[2], co = l[3]
  sg3 = sigmoid(co); m = clip(mu + coupling(sg3*x), 0, 255)
  u = exp(-sc)/2 ; n_lo = (m + 0.5 - x) ; f = clip(n_lo*u, <=43) = -t_lo/2
  E_f = e^{2f} = e^{-t_lo}; E_g = E_f*U with U = e^{-2u}  (so E_g = e^{-t_hi})
  Ep1 = 1 + E (fp32 add: replicates the reference's near-1.0 grid exactly)
  R = 1/Ep1 via 2-ULP Newton reciprocal  -> R_f = sigmoid(t_lo), R_g = sigmoid(t_hi)
  d = min(R_g - R_f, Ep1_f - Ep1_g)
The second candidate equals d*(1+E_f)(1+E_g) >= d in exact math, with the gap
vanishing exactly where it becomes the better (grid-exact) estimator: the min
needs no zone masks.  In the deep positive zone both E's round away in 1+E and
the NR reciprocal returns exactly 1.0, so d = 0 -> the 1e-12 clamp, matching
the reference's saturated-sigmoid behavior.
  e2 = max(d, 1e-12) * e1,  e1 = exp(s);  K-sums via PE (f32r, 1-pass)
  nll = ln(sum e1) - ln(sum e2).
Edge pixels (x<0.001 / x>254.999, ~13 of 1.5M) are fixed up on host.

Layout per core: partition p = (c*10+k)*4 + q (q = 16384-pixel quarter),
free dim J=1024 pixels per tile, 16 tiles.  lg/mu/co staged fp16 (sc fp32).
x replicated to all 120 partitions by stride-0 DMA: fp32 (natural channels,
for n_lo) and fp16 (source channels 0,0,1, for the coupling product).
Emission is software-pipelined (S1(t) | S2(t-1) | S3(t-2)) so every engine's
in-order queue sees ready work: DVE carries clip/AMR/reciprocal, ACT the
transcendentals, GPSIMD the tensor-tensor d-assembly, PE mu+coupling & K-sums.
"""
from contextlib import ExitStack

import numpy as np
import ml_dtypes

import concourse.bass as bass
import concourse.bacc as bacc
import concourse.tile as tile
from concourse import mybir
from concourse.bass_utils import run_bass_kernel_spmd
from concourse.dve_ops import (
    AFFINE_MUL_REDUCE,
    RECIPROCAL_APPROX_FAST,
    RECIPROCAL_APPROX_NR,
    RECIP_APPROX_FAST_CONSTS,
)

AF = mybir.ActivationFunctionType
ALU = mybir.AluOpType
F32 = mybir.dt.float32
F32R = mybir.dt.float32r
F16 = mybir.dt.float16
BF16 = mybir.dt.bfloat16

N, C, K, H, W = 8, 3, 10, 256, 256
HW = H * W            # 65536 pixels per example
CK = C * K            # 30
P = CK * 4            # 120 partitions used
NCORES = 8

J = 1024              # pixels per partition per tile
NT = HW // (4 * J)    # 16 tiles per core
QS = HW // 4          # 16384 pixel-quarter stride
LN2 = 0.6931471805599453
FCLAMP = 43.0         # keeps exp(2f) finite (exp(>=89) -> inf on ACT)


def _param_ap(t, off):
    """DRAM AP over [CK, HW] matching SBUF [120, J]: p = ck*4+q."""
    return bass.AP(tensor=t, offset=off, ap=[[HW, CK], [QS, 4], [1, J]])


def _xrep_ap(t, c, off):
    """DRAM AP over [C, HW], channel c replicated over k (3D: k, q, j)."""
    return bass.AP(tensor=t, offset=c * HW + off,
                   ap=[[0, K], [QS, 4], [1, J]])


def _out_ap(out, off):
    return bass.AP(tensor=out, offset=off, ap=[[HW, C], [QS, 4], [1, J]])


def build_kernel():
    nc = bacc.Bacc("TRN2", target_bir_lowering=False, debug=False)

    lg_d = nc.dram_tensor("lg16", [CK, HW], F16, kind="ExternalInput")
    mu_d = nc.dram_tensor("mu16", [CK, HW], F16, kind="ExternalInput")
    sc_d = nc.dram_tensor("sc32", [CK, HW], F32, kind="ExternalInput")
    co_d = nc.dram_tensor("co16", [CK, HW], F16, kind="ExternalInput")
    x_d = nc.dram_tensor("x32", [C, HW], F32, kind="ExternalInput")
    xs_d = nc.dram_tensor("xs16", [C, HW], F16, kind="ExternalInput")
    wi_d = nc.dram_tensor("w16i", [P, P], F16, kind="ExternalInput")
    wc_d = nc.dram_tensor("w16c", [P, P], F16, kind="ExternalInput")
    ws_d = nc.dram_tensor("wsum", [P, 12], F32R, kind="ExternalInput")
    out = nc.dram_tensor("out", [C, HW], F32, kind="ExternalOutput")

    with tile.TileContext(nc) as tc, ExitStack() as ctx:
        consts = ctx.enter_context(tc.tile_pool(name="consts", bufs=1))
        lpool = ctx.enter_context(tc.tile_pool(name="lpool", bufs=3))
        wk = ctx.enter_context(tc.tile_pool(name="wk", bufs=2))
        wk3 = ctx.enter_context(tc.tile_pool(name="wk3", bufs=3))
        psum = ctx.enter_context(tc.tile_pool(name="psum", bufs=2, space="PSUM"))
        stand = ctx.enter_context(tc.tile_pool(name="stand", bufs=1))

        wi = consts.tile([P, P], F16)
        wc = consts.tile([P, P], F16)
        ws = consts.tile([P, 12], F32R)
        nc.sync.dma_start(out=wi, in_=wi_d[:, :])
        nc.sync.dma_start(out=wc, in_=wc_d[:, :])
        nc.sync.dma_start(out=ws, in_=ws_d[:, :])
        bln2n = consts.tile([P, 1], F32)
        nc.vector.memset(bln2n, -LN2)
        # standing results: partition = (t-t0)*12 + (c*4+q); cols [0:J]=s1,
        # [J:2J]=s2.  Two groups so group A's ln-tail overlaps group B.
        NTA = NT // 2
        rbufA = stand.tile([NTA * 12, 2 * J], F32)
        rbufB = stand.tile([(NT - NTA) * 12, 2 * J], F32)

        rc = RECIP_APPROX_FAST_CONSTS

        def _tail(rb, t0, nt):
            nc.scalar.activation(out=rb[:, 0:J], in_=rb[:, 0:J], func=AF.Ln)
            nc.scalar.activation(out=rb[:, J:2 * J], in_=rb[:, J:2 * J],
                                 func=AF.Ln)
            nc.vector.tensor_tensor(out=rb[:, 0:J], in0=rb[:, 0:J],
                                    in1=rb[:, J:2 * J], op=ALU.subtract)
            for tt_ in range(nt):
                nc.sync.dma_start(out=_out_ap(out, (t0 + tt_) * J),
                                  in_=rb[tt_ * 12:(tt_ + 1) * 12, 0:J])

        def dma_in(t):
            lg = lpool.tile([P, J], F16, tag="lg", name="lg")
            mu = lpool.tile([P, J], F16, tag="mu", name="mu")
            sc = lpool.tile([P, J], F32, tag="sc", name="sc")
            co = lpool.tile([P, J], F16, tag="co", bufs=2, name="co")
            xr = lpool.tile([P, J], F32, tag="xr", bufs=2, name="xr")
            xs = lpool.tile([P, J], F16, tag="xs", name="xs")
            off = t * J
            nc.sync.dma_start(out=lg, in_=_param_ap(lg_d, off))
            nc.sync.dma_start(out=mu, in_=_param_ap(mu_d, off))
            nc.scalar.dma_start(out=sc, in_=_param_ap(sc_d, off))
            nc.sync.dma_start(out=co, in_=_param_ap(co_d, off))
            for c in range(C):
                nc.scalar.dma_start(out=xr[c * 40:(c + 1) * 40, :],
                                    in_=_xrep_ap(x_d, c, off))
                nc.sync.dma_start(out=xs[c * 40:(c + 1) * 40, :],
                                  in_=_xrep_ap(xs_d, c, off))
            return dict(lg=lg, mu=mu, sc=sc, co=co, xr=xr, xs=xs)

        def act_th(d):
            th = wk.tile([P, J], F16, tag="th", name="th")
            nc.scalar.activation(out=th, in_=d["co"], func=AF.Tanh, scale=0.5)
            d["th"] = th

        def s1a(t, d):
            # coupling product (fp16 2x DVE) + PE mean assembly
            q16 = wk.tile([P, J], F16, tag="q16", name="q16")
            nc.vector.tensor_tensor(out=q16, in0=d["th"], in1=d["xs"],
                                    op=ALU.mult)
            mt = psum.tile([P, J], F32, tag="pm", name="mt")
            for i in range(J // 512):
                s0, s1 = i * 512, (i + 1) * 512
                nc.tensor.matmul(mt[:, s0:s1], wi, d["mu"][:, s0:s1],
                                 start=True, stop=False)
                nc.tensor.matmul(mt[:, s0:s1], wc, q16[:, s0:s1],
                                 start=False, stop=False)
                nc.tensor.matmul(mt[:, s0:s1], wc, d["xs"][:, s0:s1],
                                 start=False, stop=True)
            return dict(t=t, mt=mt, d=d)

        def s1b(st):
            t, d, mt = st["t"], st["d"], st["mt"]
            u = wk.tile([P, J], F32, tag="u", name="u")
            nc.scalar.activation(out=u, in_=d["sc"], func=AF.Exp, scale=-1.0,
                                 bias=bln2n)
            e1 = wk3.tile([P, J], F32R, tag="e1", bufs=5, name="e1")
            nc.scalar.activation(out=e1, in_=d["lg"], func=AF.Exp)
            # cm = clip(m~, 0.5, 255.5); n_lo = cm - x on GPS (in place)
            cm = wk.tile([P, J], F32, tag="cm", name="cm")
            nc.vector.tensor_scalar(out=cm, in0=mt, scalar1=0.5,
                                    scalar2=255.5, op0=ALU.max, op1=ALU.min)
            nc.gpsimd.tensor_tensor(out=cm, in0=cm, in1=d["xr"],
                                    op=ALU.subtract)
            # f = min(n_lo*u, 43) (clamp in place); g = f - u  (g <= f <= 43)
            ab = wk.tile([P, 2 * J], F32, tag="ab", name="ab")
            nc.vector.tensor_tensor(out=ab[:, 0:J], in0=cm, in1=u, op=ALU.mult)
            nc.vector.tensor_scalar(out=ab[:, 0:J], in0=ab[:, 0:J],
                                    scalar1=FCLAMP, scalar2=None, op0=ALU.min)
            nc.gpsimd.tensor_tensor(out=ab[:, J:2 * J], in0=ab[:, 0:J],
                                    in1=u, op=ALU.subtract)
            st["ab"] = ab
            st["e1"] = e1
            return st

        def s2a(st):
            # E = exp(2*[f|g])
            E = wk3.tile([P, 2 * J], F32, tag="E", bufs=4, name="E")
            nc.scalar.activation(out=E, in_=st["ab"], func=AF.Exp, scale=2.0)
            st["E"] = E
            return st

        def s2b_act(st):
            # Ep1 = E + 1 in place, halves (exact fp32 grid)
            E = st["E"]
            nc.scalar.activation(out=E[:, 0:J], in_=E[:, 0:J],
                                 func=AF.Identity, bias=1.0)
            nc.scalar.activation(out=E[:, J:2 * J], in_=E[:, J:2 * J],
                                 func=AF.Identity, bias=1.0)

        def s2b_dve(st):
            # R = 1/Ep1 (2-ULP), J-halves for finer interleave
            E = st["E"]
            R = wk3.tile([P, 2 * J], F32, tag="R", bufs=3, name="R")
            for h in range(2):
                a, b = h * J, (h + 1) * J
                nc.vector._custom_dve(RECIPROCAL_APPROX_FAST, out=R[:, a:b],
                                      in0=E[:, a:b], s0=rc["s0"], s1=rc["s1"],
                                      imm2=rc["imm2"])
            for h in range(2):
                a, b = h * J, (h + 1) * J
                nc.vector._custom_dve(RECIPROCAL_APPROX_NR, out=R[:, a:b],
                                      in0=E[:, a:b], in1=R[:, a:b], s0=2.0)
            st["R"] = R
            return st

        def s3_gps(st):
            E, R = st["E"], st["R"]
            dpos = wk.tile([P, J], F32, tag="dpos", name="dpos")
            nc.gpsimd.tensor_tensor(out=dpos, in0=E[:, 0:J],
                                    in1=E[:, J:2 * J], op=ALU.subtract)
            s1t = wk.tile([P, J], F32, tag="s1t", name="s1t")
            nc.vector.tensor_tensor(out=s1t, in0=R[:, J:2 * J],
                                    in1=R[:, 0:J], op=ALU.subtract)
            st["s1t"] = s1t
            st["dpos"] = dpos

        def s3_rest(st):
            t, e1 = st["t"], st["e1"]
            s1t, dpos = st["s1t"], st["dpos"]
            nc.vector.tensor_tensor(out=s1t, in0=s1t, in1=dpos, op=ALU.min)
            e2 = wk.tile([P, J], F32R, tag="e2", name="e2")
            nc.vector.scalar_tensor_tensor(out=e2, in0=s1t, scalar=1e-12,
                                           in1=e1, op0=ALU.max, op1=ALU.mult)
            # PE K-sums into PSUM: r1 @ cols 0:J, r2 @ cols J:2J (partition 0)
            rp = psum.tile([12, 2 * J], F32, tag="pr", bufs=1, name="rp")
            for i in range(J // 512):
                s0, s1 = i * 512, (i + 1) * 512
                nc.tensor.matmul(rp[0:12, s0:s1], ws, e1[:, s0:s1],
                                 start=True, stop=True)
                nc.tensor.matmul(rp[0:12, J + s0:J + s1], ws, e2[:, s0:s1],
                                 start=True, stop=True)
            st["rp"] = rp

        def s4_evac(st):
            t, rp = st["t"], st["rp"]
            rsc = wk.tile([12, 2 * J], F32, tag="rsc", bufs=1, name="rsc")
            nc.scalar.copy(out=rsc, in_=rp)
            rb, tb = (rbufA, t) if t < NT // 2 else (rbufB, t - NT // 2)
            nc.sync.dma_start(out=rb[tb * 12:(tb + 1) * 12, :], in_=rsc)

        # software pipeline; stages at lags:
        #   S1a/S1b(t), E(t-1), Ep1+recip(t-2), dpos/s1(t-3),
        #   min/e2/ksum(t-4), evac(t-5)
        dmas = {0: dma_in(0), 1: dma_in(1)}
        act_th(dmas[0])
        sts = {}
        for t in range(NT + 5):
            if 0 <= t - 2 < NT:
                s2b_act(sts[t - 2])
            if t - 5 >= 0:
                s4_evac(sts.pop(t - 5))
            if 0 <= t - 3 < NT:
                s3_gps(sts[t - 3])
            if t < NT:
                if t + 2 < NT:
                    dmas[t + 2] = dma_in(t + 2)
                sts[t] = s1a(t, dmas.pop(t))
            if 0 <= t - 2 < NT:
                s2b_dve(sts[t - 2])
            if t + 1 < NT:
                act_th(dmas[t + 1])
            if t < NT:
                s1b(sts[t])
            if 0 <= t - 1 < NT:
                s2a(sts[t - 1])
            if 0 <= t - 4 < NT:
                s3_rest(sts[t - 4])
            if t - 5 == NT // 2 - 1:
                _tail(rbufA, 0, NT // 2)
        _tail(rbufB, NT // 2, NT - NT // 2)

    nc.compile()
    return nc


_CONSTS = None
_NC_CACHE = None


def _consts_np():
    global _CONSTS
    if _CONSTS is None:
        tgt = {0: 1, 1: 2, 2: 2}    # coupling target channel per coeff chan
        wi = np.eye(P, dtype=np.float16)
        wc = np.zeros((P, P), dtype=np.float16)
        for cc in range(3):
            for k in range(K):
                for q in range(4):
                    pin = (cc * K + k) * 4 + q
                    pout = (tgt[cc] * K + k) * 4 + q
                    wc[pin, pout] = 0.5
        ws = np.zeros((P, 12), dtype=np.float32)
        for c in range(C):
            for k in range(K):
                for q in range(4):
                    ws[(c * K + k) * 4 + q, c * 4 + q] = 1.0
        _CONSTS = (wi, wc, ws)
    return _CONSTS


def _host_fixup(nll, x, l):
    """Recompute edge pixels (lo_cond/hi_cond active) exactly on host."""
    f32 = np.float32
    mask = (x < f32(0.001)) | (x > f32(254.999))
    if not mask.any():
        return nll
    l6 = l.reshape(N, 4, C, K, H, W)
    with np.errstate(over="ignore"):
        sg = lambda z: (f32(1) / (f32(1) + np.exp(-z, dtype=f32))).astype(f32)
        for n, cc, hh, ww in zip(*np.nonzero(mask)):
            s = l6[n, 0, cc, :, hh, ww]
            m_raw = l6[n, 1, :, :, hh, ww]
            sc_ = np.maximum(l6[n, 2, cc, :, hh, ww], f32(-7))
            co = sg(l6[n, 3, :, :, hh, ww])
            xpix = x[n, :, hh, ww]
            if cc == 0:
                m = m_raw[0]
            elif cc == 1:
                m = (m_raw[1] + co[0] * xpix[0]).astype(f32)
            else:
                m = (m_raw[2] + co[1] * xpix[0] + co[2] * xpix[1]).astype(f32)
            m = np.clip(m, f32(0), f32(255)).astype(f32)
            cen = (xpix[cc] - m).astype(f32)
            invv = np.exp(-sc_, dtype=f32)
            lo_c = f32(1) if xpix[cc] >= f32(0.001) else f32(0)
            hi_c = f32(1) if xpix[cc] <= f32(254.999) else f32(0)
            cdf_lo = lo_c * sg(invv * (cen - f32(0.5)))
            cdf_hi = hi_c * sg(invv * (cen + f32(0.5))) + (f32(1) - hi_c)
            d = np.maximum(cdf_hi - cdf_lo, f32(1e-12))
            e1 = np.exp(s, dtype=f32)
            e2 = (e1 * d).astype(f32)
            nll[n, cc, hh, ww] = np.log(e1.sum(dtype=f32), dtype=f32) - np.log(
                e2.sum(dtype=f32), dtype=f32)
    return nll


def _get_nc():
    global _NC_CACHE
    if _NC_CACHE is None:
        _NC_CACHE = build_kernel()
    return _NC_CACHE


def _make_in_maps(x, l):
    wi, wc, ws = _consts_np()
    l6 = l.reshape(N, 4, CK, HW)
    lg16 = l6[:, 0].astype(np.float16)
    mu16 = (l6[:, 1] + np.float32(0.5)).astype(np.float16)
    sc32 = np.ascontiguousarray(l6[:, 2])
    co16 = l6[:, 3].astype(np.float16)
    x2 = x.reshape(N, C, HW)
    xs16 = x2[:, [0, 0, 1], :].astype(np.float16)  # coupling source channels
    return [
        {"lg16": lg16[n], "mu16": mu16[n], "sc32": sc32[n], "co16": co16[n],
         "x32": np.ascontiguousarray(x2[n]), "xs16": xs16[n],
         "w16i": wi, "w16c": wc, "wsum": ws}
        for n in range(NCORES)
    ]


def kernel(x, l):
    x = np.ascontiguousarray(x, dtype=np.float32)
    l = np.ascontiguousarray(l, dtype=np.float32)
    nc = _get_nc()
    in_maps = _make_in_maps(x, l)
    res = run_bass_kernel_spmd(nc, in_maps, list(range(NCORES))).results
    nll = np.stack([res[n]["out"].reshape(C, H, W) for n in range(NCORES)],
                   axis=0)
    return _host_fixup(nll, x, l)
